# revision 33
# baseline (speedup 1.0000x reference)
"""Multi-head causal attention (RoPE) on 8 Trainium2 NeuronCores.

Sharding (Megatron-style): core c handles batch c//4 and the 4 heads
[4*(c%4), 4*(c%4)+4). Each core computes Q/K/V projections for its
head slice, rotary embedding, causal flash-style attention (no
max-subtraction: scores are O(10) so exp is safe), and its partial
output projection through the matching Wo column block. The host sums
the 4 partial outputs per batch and transposes (the device computes
out.T: [model_dim, seq], bf16).

All on-device layouts are transposed ([feature, seq]). Matmul inputs
are bf16/f16 (f32 PE matmul is slower); accumulation is f32 in PSUM.

Scheduling (the per-engine instruction stream is static, so emission
order IS the schedule; measured 381us -> 313us over seven rounds):
- chunk-0 Q/K runs kt-major in two passes of 4 concurrent PSUM groups
  fed by kt-granular DMA pieces, so real matmuls start ~3us in and
  pass-A ropes overlap pass-B matmuls (~40 tiny warm matmuls cover
  the first DMA wait and the HAM cold window).
- attention is software-pipelined: QK_{j+2} plus filler matmuls are
  emitted between exp_j and AV_j so the PE never waits on the Scalar
  engine. Fillers for att(sc) = the previous chunk's Wo groups + the
  NEXT chunk's Q/K projection groups (with their ropes) + the next
  chunk's V groups, in a FIFO closure queue with lazy PSUM allocation
  (allocation at emission time keeps pool-rotation waits pointing at
  earlier-emitted instructions — no cross-engine deadlock).
- causal diagonal at 128-query granularity (band matmuls shrink
  N=512/384/256/128) with a single [128,128] triangular mask-mul.
- rope: full-partition muls on DVE + final add on GpSimd; softmax
  denominators via a ones-matmul partition-sum on the PE.
- vN/fs copies placed off the Scalar engine during attention windows
  (ACT paces them); Wo(3) fins on ACT in the tail where DVE is busy.
- out-DMA triggers alternate Sync/Scalar queues (a single queue issues
  descriptors at ~600ns each, bounding the final drain); output bf16.
"""

import os

import numpy as np
import ml_dtypes

import concourse.bass as bass
import concourse.mybir as mybir
import concourse.tile as tile
from concourse import bacc
from concourse.bass_utils import run_bass_kernel_spmd

BF16 = mybir.dt.bfloat16
F16 = mybir.dt.float16
F32 = mybir.dt.float32
NPBF16 = ml_dtypes.bfloat16
NPF16 = np.float16

NCORES = 8
B = 2
S = 2048
HDIM = 2048
NH = 16
HD = 128
HPC = 4  # heads per core
CPB = 4  # cores per batch
SCW = 512  # s-chunk width
NSC = S // SCW  # 4
KT = HDIM // 128  # 16 k-tiles
NJT = S // 128  # 16 j-tiles
SCALE = 1.0 / np.sqrt(HD)
ROPE_BASE = 10000.0

_NC_CACHE: dict[str, object] = {}
LAST_EXEC_TIME_NS = None

Exp = mybir.ActivationFunctionType.Exp
MUL = mybir.AluOpType.mult
ADD = mybir.AluOpType.add


def _build_causal():
    nc = bacc.Bacc("TRN2", target_bir_lowering=False, debug=False,
                   num_devices=NCORES)

    hst_d = nc.declare_dram_parameter("hst", [NSC, 128, KT * SCW], BF16, isOutput=False)
    wq_d = nc.declare_dram_parameter("wq", [128, KT, 512], BF16, isOutput=False)
    wk_d = nc.declare_dram_parameter("wk", [128, KT, 512], BF16, isOutput=False)
    wv_d = nc.declare_dram_parameter("wv", [128, KT * 512], BF16, isOutput=False)
    wo_d = nc.declare_dram_parameter("wo", [128, HPC * KT * 128], BF16, isOutput=False)
    cos_d = nc.declare_dram_parameter("cos2", [128, S], F32, isOutput=False)
    sin_d = nc.declare_dram_parameter("sin2", [128, S], F32, isOutput=False)
    bm_d = nc.declare_dram_parameter("bmtri", [128, 128], F16, isOutput=False)
    out_d = nc.declare_dram_parameter("outT", [HDIM, S], BF16, isOutput=True)

    with tile.TileContext(nc) as tc:
        with (
            tc.tile_pool(name="wpool", bufs=1) as wpool,
            tc.tile_pool(name="cpool", bufs=1) as cpool,
            tc.tile_pool(name="qkv", bufs=1) as qkvp,
            tc.tile_pool(name="qtp", bufs=2) as qtp,
            tc.tile_pool(name="strip", bufs=2) as stripp,
            tc.tile_pool(name="ropet", bufs=2) as ropet,
            tc.tile_pool(name="probs", bufs=4) as probsp,
            tc.tile_pool(name="pssum", bufs=2) as pssump,
            tc.tile_pool(name="recips", bufs=2) as recips,
            tc.tile_pool(name="fouts", bufs=6) as fouts,
            tc.tile_pool(name="work", bufs=3, space="PSUM") as workp,
            tc.tile_pool(name="sp", bufs=3, space="PSUM") as spp,
            tc.tile_pool(name="av", bufs=2, space="PSUM") as avp,
        ):
            wq = wpool.tile([128, KT, 512], BF16, tag="wq")  # kt-major [kt][h][fo]
            wk = wpool.tile([128, KT, 512], BF16, tag="wk")
            wv = wpool.tile([128, KT * 512], BF16, tag="wv")
            wo = wpool.tile([128, HPC * KT * 128], BF16, tag="wo")
            cos2 = cpool.tile([128, S], F32, tag="cos2")
            sin2 = cpool.tile([128, S], F32, tag="sin2")
            bmtri = cpool.tile([128, 128], F16, tag="bmtri")
            ones = cpool.tile([128, 128], F16, tag="ones")
            warm = cpool.tile([128, 128], BF16, tag="warm")
            nc.gpsimd.memset(ones[:], 1.0)
            nc.gpsimd.memset(warm[:], 0.0)

            # kTt/vN hold the full sequence (all past chunks); qT only the
            # current chunk (double-buffered); oT holds all chunks because
            # Wo(sc) is deferred into chunk sc+1 as PE filler work.
            kTt = qkvp.tile([128, HPC * S], BF16, tag="kT")
            vN = qkvp.tile([128, NJT * 512], F16, tag="vN")
            oT = qkvp.tile([128, HPC * NSC * 512], BF16, tag="oT")
            # Wo(3) h0+h1 partial, computed as filler during att(3) so
            # the post-attention tail only runs the h2+h3 half
            p01 = qkvp.tile([128, KT * 512], BF16, tag="p01")

            strips = []  # strip tiles by chunk (rotating pool, bufs=2)

            # ---- chunk-0 DMA, kt-piecewise so the PE can start early ----
            strip0 = stripp.tile([128, KT * SCW], BF16, name='strip')
            strips.append(strip0)
            # fine-grained for the first 4 kt (earliest PE start), then
            # 4-kt blocks; rope tables for chunk 0 early, the rest after
            # wv/strip1 (first needed mid-att(0) / during att(0)).
            # (strided half-width weight DMAs measured SLOWER: 512B runs
            # fragment into many packets -- keep contiguous 4-kt blocks)
            nc.sync.dma_start(strip0[:, 0:4 * SCW], hst_d[0][:, 0:4 * SCW])
            nc.sync.dma_start(wq[:, 0:4, :], wq_d[:, 0:4, :])
            nc.sync.dma_start(wk[:, 0:4, :], wk_d[:, 0:4, :])
            nc.sync.dma_start(cos2[:, 0:SCW], cos_d[:, 0:SCW])
            nc.sync.dma_start(sin2[:, 0:SCW], sin_d[:, 0:SCW])
            for piece in range(1, 4):
                k0 = piece * 4
                nc.sync.dma_start(
                    strip0[:, k0 * SCW:(k0 + 4) * SCW],
                    hst_d[0][:, k0 * SCW:(k0 + 4) * SCW],
                )
                nc.sync.dma_start(wq[:, k0:k0 + 4, :], wq_d[:, k0:k0 + 4, :])
                nc.sync.dma_start(wk[:, k0:k0 + 4, :], wk_d[:, k0:k0 + 4, :])
            nc.sync.dma_start(wv[:], wv_d[:])
            strip1 = stripp.tile([128, KT * SCW], BF16, name='strip')
            strips.append(strip1)
            nc.sync.dma_start(strip1[:], hst_d[1])
            nc.sync.dma_start(cos2[:, SCW:S], cos_d[:, SCW:S])
            nc.sync.dma_start(sin2[:, SCW:S], sin_d[:, SCW:S])
            nc.sync.dma_start(bmtri[:], bm_d[:])
            nc.sync.dma_start(wo[:], wo_d[:])

            # ---- tiny PE warmup: fill the ~3us DMA wait, warm the HAM ----
            wps = avp.tile([128, 512], F32, name='a')
            for _ in range(40):
                nc.tensor.matmul(wps[:, 0:128], warm[:], warm[:],
                                 start=True, stop=True)

            # ================= emission helpers =================

            filler_q: list = []  # list of closures, each emits 1 PE matmul

            def emit_v_group(sc, st):
                """V projection group for jt = 4*sc+st: 16 accumulating
                matmuls + an ACT copy to vN. Emits everything now."""
                strip = strips[sc]
                vp = workp.tile([128, 512], F32, name='w')
                jt = sc * 4 + st
                for kt in range(KT):
                    nc.tensor.matmul(
                        vp[:],
                        strip[:, kt * SCW + st * 128: kt * SCW + (st + 1) * 128],
                        wv[:, kt * 512:(kt + 1) * 512],
                        start=(kt == 0), stop=(kt == KT - 1),
                    )
                # DVE, not ACT: the ACT queue paces attention windows and
                # a copy there delays exps -> AV matmuls wait on vN.
                nc.vector.tensor_copy(vN[:, jt * 512:(jt + 1) * 512], vp[:])

            def push_wo_group(sc, mt, fin_on_act=False):
                """Wo block mt for chunk sc: 4 accumulating matmuls +
                copy + out-DMA, as lazily-allocating filler closures."""
                cell: list = []  # holds fp once the first closure runs
                cl = []
                for h in range(HPC):
                    def mm(h=h, sc=sc, mt=mt, cell=cell):
                        if not cell:
                            cell.append(workp.tile([128, 512], F32, name='w'))
                        nc.tensor.matmul(
                            cell[0][:],
                            wo[:, (h * KT + mt) * 128:(h * KT + mt + 1) * 128],
                            oT[:, (h * NSC + sc) * 512:(h * NSC + sc + 1) * 512],
                            start=(h == 0), stop=(h == HPC - 1),
                        )
                    cl.append(mm)

                def fin(sc=sc, mt=mt, cell=cell, fin_on_act=fin_on_act):
                    fs = fouts.tile([128, 512], BF16, name='fs')
                    # DVE during attention windows (ACT paces them); ACT
                    # for the tail Wo(3), where DVE still has att cleanup
                    # queued and ACT is done with exps
                    if fin_on_act:
                        nc.scalar.copy(fs[:], cell[0][:])
                    else:
                        nc.vector.tensor_copy(fs[:], cell[0][:])
                    # alternate DMA trigger queues: a single queue issues
                    # descriptors at ~600ns each, which bounds the final
                    # out-DMA drain after the last matmul
                    dq = nc.scalar if mt % 2 else nc.sync
                    dq.dma_start(
                        out_d[mt * 128:(mt + 1) * 128, sc * SCW:(sc + 1) * SCW],
                        fs[:],
                    )
                cl.append(fin)
                filler_q.extend(cl)

            def push_wo3_phase1(mt):
                """Wo(3) block mt, heads 0+1 only: 2 matmuls + a bf16
                partial copy to SBUF. Drains during att(3) h2/h3."""
                cell: list = []

                def mm(h, mt=mt, cell=cell):
                    if not cell:
                        cell.append(workp.tile([128, 512], F32, name='w'))
                    nc.tensor.matmul(
                        cell[0][:],
                        wo[:, (h * KT + mt) * 128:(h * KT + mt + 1) * 128],
                        oT[:, (h * NSC + NSC - 1) * 512:
                           (h * NSC + NSC) * 512],
                        start=(h == 0), stop=(h == 1),
                    )

                def cp(mt=mt, cell=cell):
                    nc.vector.tensor_copy(
                        p01[:, mt * 512:(mt + 1) * 512], cell[0][:])
                filler_q.append(lambda: mm(0))
                filler_q.append(lambda: mm(1))
                filler_q.append(cp)

            def drain_fillers(n):
                for _ in range(n):
                    if filler_q:
                        filler_q.pop(0)()

            def rope(pq, dst, sc):
                """dst = pq*cos + rotate_half(pq)*sin for chunk sc.
                pq: [128,512] PSUM f32; dst: [128,512] SBUF bf16 slice."""
                cs = cos2[:, sc * SCW:(sc + 1) * SCW]
                sn_lo = sin2[0:64, sc * SCW:(sc + 1) * SCW]    # -sin
                sn_hi = sin2[64:128, sc * SCW:(sc + 1) * SCW]  # +sin
                t1 = ropet.tile([128, SCW], F32, name='rt')
                t2 = ropet.tile([128, SCW], F32, name='rt')
                nc.vector.tensor_mul(t1[:], pq[:], cs)
                nc.vector.tensor_mul(t2[0:64, :], pq[64:128, :], sn_lo)
                nc.vector.tensor_mul(t2[64:128, :], pq[0:64, :], sn_hi)
                nc.gpsimd.tensor_add(dst, t1[:], t2[:])

            def attention_head(sc, h, qT, emit_v=False, drain_pat=(2,)):
                """Software-pipelined attention for (chunk sc, head h).
                j-tiles: nfull = 4*sc full [128k,512q] tiles, then 4 band
                tiles with N = 512-128t. Emits fillers between steps.
                emit_v: emit V(sc) group st=j as a block after full step j
                (head 0 of chunks >= 1) so vN is ready for the band."""
                nfull = 4 * sc
                nj = nfull + 4
                qbase = h * SCW

                def qk(j):
                    if j < nfull:
                        n = SCW
                        rhs = qT[:, qbase:qbase + SCW]
                    else:
                        t = j - nfull
                        n = SCW - 128 * t
                        rhs = qT[:, qbase + 128 * t:qbase + SCW]
                    sp = spp.tile([128, SCW], F32, name='s')
                    jj = j  # absolute key tile index == j (tiles 0..nj-1)
                    nc.tensor.matmul(
                        sp[:, 0:n],
                        kTt[:, h * S + jj * 128:h * S + (jj + 1) * 128],
                        rhs, start=True, stop=True,
                    )
                    return sp, n

                av = avp.tile([128, SCW], F32, name='a')
                Ps = pssump.tile([128, SCW], F16, name='Ps')
                sps = {}
                sps[0] = qk(0)
                if nj > 1:
                    sps[1] = qk(1)
                for j in range(nj):
                    sp, n = sps.pop(j)
                    t = j - nfull  # >=0 for band tiles
                    if j == 0:
                        pr = Ps  # exp writes the running-sum tile directly
                        nc.scalar.activation(pr[:, 0:n], sp[:, 0:n], Exp,
                                             scale=float(SCALE))
                    else:
                        pr = probsp.tile([128, SCW], F16, name='pr')
                        nc.scalar.activation(pr[:, 0:n], sp[:, 0:n], Exp,
                                             scale=float(SCALE))
                    if t >= 0:
                        # mask the triangular diagonal sub-block
                        nc.vector.tensor_mul(pr[:, 0:128], pr[:, 0:128],
                                             bmtri[:])
                    if j > 0:
                        c0 = SCW - n
                        nc.vector.tensor_add(Ps[:, c0:SCW], Ps[:, c0:SCW],
                                             pr[:, 0:n])
                    if j + 2 < nj:
                        sps[j + 2] = qk(j + 2)
                    if emit_v and j < 3:
                        # V group j+1 (group j was emitted before this
                        # head's first step / previous iteration)
                        emit_v_group(sc, j + 1)
                    drain_fillers(drain_pat[j % len(drain_pat)])
                    c0 = SCW - n
                    nc.tensor.matmul(
                        av[:, c0:SCW],
                        vN[:, j * 512 + h * 128:j * 512 + (h + 1) * 128],
                        pr[:, 0:n],
                        start=(j == 0), stop=(j == nj - 1),
                    )
                # normalization: partition-sum via ones-matmul (broadcasts
                # the denominator across partitions), then reciprocal.
                drain_fillers(2)
                rs = workp.tile([128, 512], F32, name='w')
                nc.tensor.matmul(rs[:], ones[:], Ps[:], start=True, stop=True)
                rc = recips.tile([128, SCW], F32, name='rc')
                nc.vector.reciprocal_approx_fast(rc[:], rs[:])
                nc.vector.tensor_mul(
                    oT[:, (h * NSC + sc) * 512:(h * NSC + sc + 1) * 512],
                    av[:], rc[:],
                )

            # ================= chunk 0: kt-major Q/K =================
            # Two passes of 4 concurrent PSUM groups: pass-A (heads 0,1)
            # finishes ~halfway so its ropes run on DVE/GpSimd while the
            # PE streams pass-B, instead of one rope burst at the end.
            qT0 = qtp.tile([128, HPC * SCW], BF16, name='qT')
            for half in range(2):
                banks = []
                pools = ((workp, 'w'), (workp, 'w'), (workp, 'w'), (spp, 's')) \
                    if half == 0 else \
                    ((spp, 's'), (spp, 's'), (spp, 's'), (avp, 'a'))
                for pool, nm in pools:
                    banks.append(pool.tile([128, 512], F32, name=nm))
                for kt in range(KT):
                    for i in range(4):
                        h = 2 * half + i // 2
                        wt = wk if i % 2 else wq
                        nc.tensor.matmul(
                            banks[i][:],
                            wt[:, kt, h * 128:(h + 1) * 128],
                            strip0[:, kt * SCW:(kt + 1) * SCW],
                            start=(kt == 0), stop=(kt == KT - 1),
                        )
                for i in range(4):
                    h = 2 * half + i // 2
                    if i % 2:
                        rope(banks[i], kTt[:, h * S:h * S + SCW], 0)
                    else:
                        rope(banks[i], qT0[:, h * SCW:(h + 1) * SCW], 0)

            def push_qk_group(sc, h, isk, qt_cell):
                """Q or K projection group for chunk sc as 16 lazily-
                allocating filler closures + a rope closure."""
                strip = strips[sc]
                cell: list = []
                for kt in range(KT):
                    def mm(kt=kt, h=h, isk=isk, cell=cell, strip=strip):
                        if not cell:
                            cell.append(workp.tile([128, 512], F32, name='w'))
                        wt = wk if isk else wq
                        nc.tensor.matmul(
                            cell[0][:],
                            wt[:, kt, h * 128:(h + 1) * 128],
                            strip[:, kt * SCW:(kt + 1) * SCW],
                            start=(kt == 0), stop=(kt == KT - 1),
                        )
                    filler_q.append(mm)

                def rp(h=h, isk=isk, cell=cell, sc=sc, qt_cell=qt_cell):
                    if isk:
                        rope(cell[0], kTt[:, h * S + sc * SCW:h * S + (sc + 1) * SCW], sc)
                    else:
                        if not qt_cell:
                            qt_cell.append(
                                qtp.tile([128, HPC * SCW], BF16, name='qT'))
                        rope(cell[0], qt_cell[0][:, h * SCW:(h + 1) * SCW], sc)
                filler_q.append(rp)

            def push_chunk_fillers(next_sc, wo_sc):
                """Interleave Wo(wo_sc), QK(next_sc) and V(next_sc) groups
                in the filler queue: [Wog, Wog, QKg, (Vg)] x 8. Wo first:
                it gives the prefetched strip DMA a head start before QK
                matmuls (which wait on it) enter the static PE stream.
                V(next_sc) here means its vN copies land well before
                att(next_sc)'s band AV matmuls need them."""
                qt_cell: list = []
                wo_mt = 0
                for g in range(8):
                    if wo_sc is not None:
                        push_wo_group(wo_sc, wo_mt)
                    if next_sc is not None and g < 4:
                        # V groups early: their vN copies (DVE) must land
                        # before att(next_sc) h0's band AV matmuls
                        filler_q.append(
                            lambda sc=next_sc, st=g: emit_v_group(sc, st))
                    if wo_sc is not None:
                        push_wo_group(wo_sc, wo_mt + 1)
                        wo_mt += 2
                    if next_sc is not None:
                        push_qk_group(next_sc, g // 2, g % 2, qt_cell)
                return qt_cell

            # att(0), with QK(1) draining as fillers between steps
            qt_next = push_chunk_fillers(1, None)
            for h in range(HPC):
                if h == 0:
                    emit_v_group(0, 0)
                attention_head(0, h, qT0, emit_v=(h == 0))
            drain_fillers(len(filler_q))

            # ================= chunks 1..3 =================
            for sc in range(1, NSC):
                qT = qt_next[0]
                if sc + 1 < NSC:
                    stn = stripp.tile([128, KT * SCW], BF16, name='strip')
                    strips.append(stn)
                    for k0 in range(0, KT, 4):  # piecewise: QK fillers
                        nc.sync.dma_start(     # wait per-piece, not 2MB
                            stn[:, k0 * SCW:(k0 + 4) * SCW],
                            hst_d[sc + 1][:, k0 * SCW:(k0 + 4) * SCW],
                        )
                    qt_next = push_chunk_fillers(sc + 1, sc - 1)
                else:
                    push_chunk_fillers(None, sc - 1)
                # V(sc) was already pushed/drained during att(sc-1).
                # Last chunk: the queue (only Wo(2), 80 closures) would
                # drain out mid-head-2 at 2/step, leaving head 3 with no
                # PE filler against the ACT-paced exps — ration it.
                last = sc == NSC - 1
                for h in range(HPC):
                    # last chunk: Wo(2)'s 80 closures cover h0/h1 at
                    # ~1.25/step; Wo(3)'s h0+h1 phase (pushed once those
                    # heads' oT is emitted) covers h2/h3 at 2/step
                    pat = (1, 1, 1, 2) if (last and h < 2) else (2,)
                    attention_head(sc, h, qT, drain_pat=pat)
                    if last and h == 1:
                        for mt in range(KT):
                            push_wo3_phase1(mt)
                drain_fillers(len(filler_q))

            # tail: Wo(3) heads 2+3 only, combined with the p01 partial
            for mt in range(KT):
                fp = workp.tile([128, 512], F32, name='w')
                for h in (2, 3):
                    nc.tensor.matmul(
                        fp[:],
                        wo[:, (h * KT + mt) * 128:(h * KT + mt + 1) * 128],
                        oT[:, (h * NSC + NSC - 1) * 512:
                           (h * NSC + NSC) * 512],
                        start=(h == 2), stop=(h == 3),
                    )
                fs = fouts.tile([128, 512], BF16, name='fs')
                nc.vector.scalar_tensor_tensor(
                    fs[:], fp[:], 1.0, p01[:, mt * 512:(mt + 1) * 512],
                    MUL, ADD)
                dq = nc.scalar if mt % 2 else nc.sync
                dq.dma_start(
                    out_d[mt * 128:(mt + 1) * 128,
                          (NSC - 1) * SCW:NSC * SCW],
                    fs[:],
                )

    nc.compile()
    return nc


def _build_fallback(mode: str):
    """mode: 'full' | 'general' — the original (baseline) kernel."""
    nc = bacc.Bacc("TRN2", target_bir_lowering=False, debug=False,
                   num_devices=NCORES)

    hst_d = nc.declare_dram_parameter("hst", [NSC, 128, KT * SCW], BF16, isOutput=False)
    wq_d = nc.declare_dram_parameter("wq", [128, HPC * KT * 128], BF16, isOutput=False)
    wk_d = nc.declare_dram_parameter("wk", [128, HPC * KT * 128], BF16, isOutput=False)
    wv_d = nc.declare_dram_parameter("wv", [128, KT * 512], BF16, isOutput=False)
    wo_d = nc.declare_dram_parameter("wo", [128, HPC * KT * 128], BF16, isOutput=False)
    cos_d = nc.declare_dram_parameter("cost", [64, S], F32, isOutput=False)
    sin_d = nc.declare_dram_parameter("sint", [64, S], F32, isOutput=False)
    if mode == "general":
        em_d = nc.declare_dram_parameter("emask", [S, S], F16, isOutput=False)
    out_d = nc.declare_dram_parameter("outT", [HDIM, S], F32, isOutput=True)

    with tile.TileContext(nc) as tc:
        with (
            tc.tile_pool(name="wpool", bufs=1) as wpool,
            tc.tile_pool(name="cpool", bufs=1) as cpool,
            tc.tile_pool(name="qkv", bufs=1) as qkvp,
            tc.tile_pool(name="strip", bufs=2) as stripp,
            tc.tile_pool(name="ropet", bufs=2) as ropet,
            tc.tile_pool(name="probs", bufs=4) as probsp,
            tc.tile_pool(name="psums", bufs=2) as psums,
            tc.tile_pool(name="recips", bufs=2) as recips,
            tc.tile_pool(name="fouts", bufs=3) as fouts,
            tc.tile_pool(name="emt", bufs=4) as emtp,
            tc.tile_pool(name="pp", bufs=2, space="PSUM") as pp_pool,
            tc.tile_pool(name="sp", bufs=2, space="PSUM") as sp_pool,
            tc.tile_pool(name="av", bufs=2, space="PSUM") as av_pool,
            tc.tile_pool(name="misc_ps", bufs=2, space="PSUM") as misc_ps,
        ):
            wq = wpool.tile([128, HPC * KT * 128], BF16, tag="wq")
            wk = wpool.tile([128, HPC * KT * 128], BF16, tag="wk")
            wv = wpool.tile([128, KT * 512], BF16, tag="wv")
            wo = wpool.tile([128, HPC * KT * 128], BF16, tag="wo")
            cos = cpool.tile([64, S], F32, tag="cos")
            sin = cpool.tile([64, S], F32, tag="sin")
            ones = cpool.tile([128, 128], F16, tag="ones")
            nc.gpsimd.memset(ones[:], 1.0)

            qT = qkvp.tile([128, HPC * S], BF16, tag="qT")
            kTt = qkvp.tile([128, HPC * S], BF16, tag="kT")
            vN = qkvp.tile([128, NJT * 512], F16, tag="vN")
            oT = qkvp.tile([128, HPC * NSC * 512], BF16, tag="oT")
            # Wo(3) h0+h1 partial, computed as filler during att(3) so
            # the post-attention tail only runs the h2+h3 half
            p01 = qkvp.tile([128, KT * 512], BF16, tag="p01")

            warm = cpool.tile([128, 512], BF16, tag="warm")
            nc.gpsimd.memset(warm[:, 0:128], 0.0)
            wps = misc_ps.tile([128, SCW], F32, tag="mp")
            for _ in range(100):
                nc.tensor.matmul(wps[:], warm[:, 0:128], warm[:],
                                 start=True, stop=True)

            for sc in range(NSC):
                strip = stripp.tile([128, KT * SCW], BF16)
                nc.sync.dma_start(strip[:], hst_d[sc])
                if sc == 0:
                    nc.sync.dma_start(wq[:], wq_d[:])
                    nc.sync.dma_start(cos[:], cos_d[:])
                    nc.sync.dma_start(sin[:], sin_d[:])
                    nc.sync.dma_start(wk[:], wk_d[:])
                    nc.sync.dma_start(wv[:], wv_d[:])
                    nc.sync.dma_start(wo[:], wo_d[:])
                cs = cos[:, sc * SCW:(sc + 1) * SCW]
                sn = sin[:, sc * SCW:(sc + 1) * SCW]
                for h in range(HPC):
                    for wt, dst in ((wq, qT), (wk, kTt)):
                        pq = pp_pool.tile([128, SCW], F32, tag="pp")
                        for kt in range(KT):
                            nc.tensor.matmul(
                                pq[:],
                                wt[:, (h * KT + kt) * 128:(h * KT + kt + 1) * 128],
                                strip[:, kt * SCW:(kt + 1) * SCW],
                                start=(kt == 0), stop=(kt == KT - 1),
                            )
                        dlo = dst[0:64, h * S + sc * SCW: h * S + (sc + 1) * SCW]
                        dhi = dst[64:128, h * S + sc * SCW: h * S + (sc + 1) * SCW]
                        t1 = ropet.tile([128, SCW], F32, tag="t1")
                        t2 = ropet.tile([128, SCW], F32, tag="t2")
                        nc.vector.tensor_mul(t1[0:64, :], pq[0:64, :], cs)
                        nc.vector.tensor_mul(t1[64:128, :], pq[64:128, :], cs)
                        nc.vector.tensor_mul(t2[0:64, :], pq[64:128, :], sn)
                        nc.vector.tensor_mul(t2[64:128, :], pq[0:64, :], sn)
                        nc.vector.tensor_sub(dlo, t1[0:64, :], t2[0:64, :])
                        nc.vector.tensor_add(dhi, t1[64:128, :], t2[64:128, :])
                for st in range(4):
                    vp = pp_pool.tile([128, SCW], F32, tag="pp")
                    for kt in range(KT):
                        nc.tensor.matmul(
                            vp[:],
                            strip[:, kt * SCW + st * 128: kt * SCW + (st + 1) * 128],
                            wv[:, kt * 512:(kt + 1) * 512],
                            start=(kt == 0), stop=(kt == KT - 1),
                        )
                    jt = sc * 4 + st
                    nc.scalar.copy(vN[:, jt * 512:(jt + 1) * 512], vp[:])

                ics = list(range(NSC)) if sc == NSC - 1 else []
                for ic in ics:
                    nj = NJT
                    for h in range(HPC):
                        av = av_pool.tile([128, SCW], F32)
                        Ps = psums.tile([128, SCW], F16)
                        for j in range(nj):
                            sp = sp_pool.tile([128, SCW], F32)
                            nc.tensor.matmul(
                                sp[:],
                                kTt[:, h * S + j * 128: h * S + (j + 1) * 128],
                                qT[:, h * S + ic * SCW: h * S + (ic + 1) * SCW],
                                start=True, stop=True,
                            )
                            pr = probsp.tile([128, SCW], F16)
                            nc.scalar.activation(pr[:], sp[:], Exp, scale=float(SCALE))
                            if mode == "general":
                                emt = emtp.tile([128, SCW], F16)
                                nc.sync.dma_start(
                                    emt[:],
                                    em_d[j * 128:(j + 1) * 128, ic * SCW:(ic + 1) * SCW],
                                )
                                nc.vector.tensor_mul(pr[:], pr[:], emt[:])
                            if j == 0:
                                nc.vector.tensor_copy(Ps[:], pr[:])
                            else:
                                nc.vector.tensor_add(Ps[:], Ps[:], pr[:])
                            nc.tensor.matmul(
                                av[:],
                                vN[:, j * 512 + h * 128: j * 512 + (h + 1) * 128],
                                pr[:],
                                start=(j == 0), stop=(j == nj - 1),
                            )
                        rs = misc_ps.tile([128, SCW], F32, tag="mp")
                        nc.tensor.matmul(rs[:], ones[:], Ps[:], start=True, stop=True)
                        rc = recips.tile([128, SCW], F32)
                        nc.vector.reciprocal_approx_fast(rc[:], rs[:])
                        nc.vector.tensor_mul(
                            oT[:, (h * NSC + ic) * 512:(h * NSC + ic + 1) * 512],
                            av[:], rc[:],
                        )
                    for mt in range(KT):
                        fp = misc_ps.tile([128, SCW], F32, tag="mp")
                        for h in range(HPC):
                            nc.tensor.matmul(
                                fp[:],
                                wo[:, (h * KT + mt) * 128:(h * KT + mt + 1) * 128],
                                oT[:, (h * NSC + ic) * 512:(h * NSC + ic + 1) * 512],
                                start=(h == 0), stop=(h == HPC - 1),
                            )
                        fs = fouts.tile([128, SCW], F32)
                        nc.scalar.copy(fs[:], fp[:])
                        nc.sync.dma_start(
                            out_d[mt * 128:(mt + 1) * 128, ic * SCW:(ic + 1) * SCW],
                            fs[:],
                        )

    nc.compile()
    return nc


def _get_nc(mode: str):
    if mode not in _NC_CACHE:
        if mode == "causal":
            _NC_CACHE[mode] = _build_causal()
        else:
            _NC_CACHE[mode] = _build_fallback(mode)
    return _NC_CACHE[mode]


def _classify_mask(m: np.ndarray) -> str:
    if not m.any():
        return "full"
    tril = np.tril(np.ones((S, S), dtype=bool))
    if np.all(m[tril] == 0.0) and np.all(m[~tril] <= -1e8):
        return "causal"
    return "general"


def kernel(hidden_states, attention_mask, position_ids, Wq, Wk, Wv, Wo):
    global LAST_EXEC_TIME_NS
    hs = np.asarray(hidden_states, dtype=np.float32)
    mask = np.asarray(attention_mask, dtype=np.float32)[0, 0]
    pos = np.asarray(position_ids)
    Wq = np.asarray(Wq, dtype=np.float32)
    Wk = np.asarray(Wk, dtype=np.float32)
    Wv = np.asarray(Wv, dtype=np.float32)
    Wo = np.asarray(Wo, dtype=np.float32)

    mode = _classify_mask(mask)
    nc = _get_nc(mode)

    inv_freq = 1.0 / (ROPE_BASE ** (np.arange(0, HD, 2, dtype=np.float32) / HD))
    cos_b, sin_b = [], []
    for b in range(B):
        ang = np.outer(pos[b].astype(np.float32), inv_freq)  # [S, 64]
        cos_b.append(np.cos(ang).T.astype(np.float32).copy())  # [64, S]
        sin_b.append(np.sin(ang).T.astype(np.float32).copy())

    in_maps = []
    for c in range(NCORES):
        b = c // CPB
        r0 = (c % CPB) * HPC * HD

        hsb = hs[b]  # [S, HDIM]
        hst = (hsb.reshape(NSC, SCW, KT, 128).transpose(0, 3, 2, 1)
               .reshape(NSC, 128, KT * SCW).astype(NPBF16))

        Wv_s = Wv[r0:r0 + 512]
        wv_t = (Wv_s.reshape(512, KT, 128).transpose(2, 1, 0)
                .reshape(128, KT * 512).astype(NPBF16))
        Wo_s = Wo[:, r0:r0 + 512]
        wo_t = (Wo_s.reshape(KT, 128, HPC, 128).transpose(3, 2, 0, 1)
                .reshape(128, HPC * KT * 128).astype(NPBF16))

        if mode == "causal":
            # flat kt-major weight layout [ki, kt*512 + h*128 + fo] —
            # exactly the SBUF tile layout, so DMA slices are contiguous
            Wq_s = Wq[r0:r0 + 512]
            wq_t = np.ascontiguousarray(
                Wq_s.reshape(HPC, 128, KT, 128).transpose(3, 2, 0, 1)
                .reshape(128, KT, 512)).astype(NPBF16)
            Wk_s = Wk[r0:r0 + 512]
            wk_t = np.ascontiguousarray(
                Wk_s.reshape(HPC, 128, KT, 128).transpose(3, 2, 0, 1)
                .reshape(128, KT, 512)).astype(NPBF16)
            cos2 = np.concatenate([cos_b[b], cos_b[b]], axis=0)  # [128, S]
            sin2 = np.concatenate([-sin_b[b], sin_b[b]], axis=0)
            pidx = np.arange(128)[:, None]
            xidx = np.arange(128)[None, :]
            bmtri = (pidx <= xidx).astype(NPF16)
            m = {
                "hst": hst, "wq": wq_t, "wk": wk_t, "wv": wv_t, "wo": wo_t,
                "cos2": np.ascontiguousarray(cos2),
                "sin2": np.ascontiguousarray(sin2),
                "bmtri": bmtri,
            }
        else:
            Wq_s = Wq[r0:r0 + 512]
            wq_t = (Wq_s.reshape(HPC, 128, KT, 128).transpose(3, 0, 2, 1)
                    .reshape(128, HPC * KT * 128).astype(NPBF16))
            Wk_s = Wk[r0:r0 + 512]
            wk_t = (Wk_s.reshape(HPC, 128, KT, 128).transpose(3, 0, 2, 1)
                    .reshape(128, HPC * KT * 128).astype(NPBF16))
            m = {
                "hst": hst, "wq": wq_t, "wk": wk_t, "wv": wv_t, "wo": wo_t,
                "cost": cos_b[b], "sint": sin_b[b],
            }
            if mode == "general":
                with np.errstate(under="ignore", over="ignore"):
                    m["emask"] = np.exp(mask.T.astype(np.float64)).astype(NPF16)
        in_maps.append(m)

    trace = os.environ.get("BASS_KERNEL_TRACE") == "1"
    res = run_bass_kernel_spmd(nc, in_maps, core_ids=list(range(NCORES)),
                               trace=trace)
    LAST_EXEC_TIME_NS = res.exec_time_ns

    out = np.empty((B, S, HDIM), dtype=np.float32)
    for b in range(B):
        acc = res.results[CPB * b]["outT"].astype(np.float32)
        for c in range(CPB * b + 1, CPB * (b + 1)):
            acc = acc + res.results[c]["outT"].astype(np.float32)
        out[b] = acc.T
    return out


# revision 35
# speedup vs baseline: 1.0111x; 1.0111x over previous
"""Multi-head causal attention (RoPE) on 8 Trainium2 NeuronCores.

Sharding (Megatron-style): core c handles batch c//4 and the 4 heads
[4*(c%4), 4*(c%4)+4). Each core computes Q/K/V projections for its
head slice, rotary embedding, causal flash-style attention (no
max-subtraction: scores are O(10) so exp is safe), and its partial
output projection through the matching Wo column block. The host sums
the 4 partial outputs per batch and transposes (the device computes
out.T: [model_dim, seq], bf16).

All on-device layouts are transposed ([feature, seq]). Matmul inputs
are bf16/f16 (f32 PE matmul is slower); accumulation is f32 in PSUM.

Scheduling (the per-engine instruction stream is static, so emission
order IS the schedule; measured 381us -> 313us over seven rounds):
- chunk-0 Q/K runs kt-major in two passes of 4 concurrent PSUM groups
  fed by kt-granular DMA pieces, so real matmuls start ~3us in and
  pass-A ropes overlap pass-B matmuls (~40 tiny warm matmuls cover
  the first DMA wait and the HAM cold window).
- attention is software-pipelined: QK_{j+2} plus filler matmuls are
  emitted between exp_j and AV_j so the PE never waits on the Scalar
  engine. Fillers for att(sc) = the previous chunk's Wo groups + the
  NEXT chunk's Q/K projection groups (with their ropes) + the next
  chunk's V groups, in a FIFO closure queue with lazy PSUM allocation
  (allocation at emission time keeps pool-rotation waits pointing at
  earlier-emitted instructions — no cross-engine deadlock).
- causal diagonal at 128-query granularity (band matmuls shrink
  N=512/384/256/128) with a single [128,128] triangular mask-mul.
- rope: full-partition muls on DVE + final add on GpSimd; softmax
  denominators via a ones-matmul partition-sum on the PE.
- vN/fs copies placed off the Scalar engine during attention windows
  (ACT paces them); Wo(3) fins on ACT in the tail where DVE is busy.
- out-DMA triggers alternate Sync/Scalar queues (a single queue issues
  descriptors at ~600ns each, bounding the final drain); output bf16.
"""

import os

import numpy as np
import ml_dtypes

import concourse.bass as bass
import concourse.mybir as mybir
import concourse.tile as tile
from concourse import bacc
from concourse.bass_utils import run_bass_kernel_spmd

BF16 = mybir.dt.bfloat16
F16 = mybir.dt.float16
F32 = mybir.dt.float32
NPBF16 = ml_dtypes.bfloat16
NPF16 = np.float16

NCORES = 8
B = 2
S = 2048
HDIM = 2048
NH = 16
HD = 128
HPC = 4  # heads per core
CPB = 4  # cores per batch
SCW = 512  # s-chunk width
NSC = S // SCW  # 4
KT = HDIM // 128  # 16 k-tiles
NJT = S // 128  # 16 j-tiles
SCALE = 1.0 / np.sqrt(HD)
ROPE_BASE = 10000.0

_NC_CACHE: dict[str, object] = {}
LAST_EXEC_TIME_NS = None

Exp = mybir.ActivationFunctionType.Exp
MUL = mybir.AluOpType.mult
ADD = mybir.AluOpType.add


def _build_causal():
    nc = bacc.Bacc("TRN2", target_bir_lowering=False, debug=False,
                   num_devices=NCORES)

    hst_d = nc.declare_dram_parameter("hst", [NSC, 128, KT * SCW], BF16, isOutput=False)
    wq_d = nc.declare_dram_parameter("wq", [128, KT, 512], BF16, isOutput=False)
    wk_d = nc.declare_dram_parameter("wk", [128, KT, 512], BF16, isOutput=False)
    wv_d = nc.declare_dram_parameter("wv", [128, KT * 512], BF16, isOutput=False)
    wo_d = nc.declare_dram_parameter("wo", [128, HPC * KT * 128], BF16, isOutput=False)
    cos_d = nc.declare_dram_parameter("cos2", [128, S], F32, isOutput=False)
    sin_d = nc.declare_dram_parameter("sin2", [128, S], F32, isOutput=False)
    bm_d = nc.declare_dram_parameter("bmtri", [128, 128], F16, isOutput=False)
    out_d = nc.declare_dram_parameter("outT", [HDIM, S], BF16, isOutput=True)

    with tile.TileContext(nc) as tc:
        with (
            tc.tile_pool(name="wpool", bufs=1) as wpool,
            tc.tile_pool(name="cpool", bufs=1) as cpool,
            tc.tile_pool(name="qkv", bufs=1) as qkvp,
            tc.tile_pool(name="qtp", bufs=2) as qtp,
            tc.tile_pool(name="strip", bufs=2) as stripp,
            tc.tile_pool(name="ropet", bufs=4) as ropet,
            tc.tile_pool(name="probs", bufs=5) as probsp,
            tc.tile_pool(name="pssum", bufs=2) as pssump,
            tc.tile_pool(name="recips", bufs=2) as recips,
            tc.tile_pool(name="fouts", bufs=8) as fouts,
            tc.tile_pool(name="work", bufs=3, space="PSUM") as workp,
            tc.tile_pool(name="sp", bufs=3, space="PSUM") as spp,
            tc.tile_pool(name="av", bufs=2, space="PSUM") as avp,
        ):
            wq = wpool.tile([128, KT, 512], BF16, tag="wq")  # kt-major [kt][h][fo]
            wk = wpool.tile([128, KT, 512], BF16, tag="wk")
            wv = wpool.tile([128, KT * 512], BF16, tag="wv")
            wo = wpool.tile([128, HPC * KT * 128], BF16, tag="wo")
            cos2 = cpool.tile([128, S], F32, tag="cos2")
            sin2 = cpool.tile([128, S], F32, tag="sin2")
            bmtri = cpool.tile([128, 128], F16, tag="bmtri")
            ones = cpool.tile([128, 128], F16, tag="ones")
            warm = cpool.tile([128, 128], BF16, tag="warm")
            nc.gpsimd.memset(ones[:], 1.0)
            nc.gpsimd.memset(warm[:], 0.0)

            # kTt/vN hold the full sequence (all past chunks); qT only the
            # current chunk (double-buffered); oT holds all chunks because
            # Wo(sc) is deferred into chunk sc+1 as PE filler work.
            kTt = qkvp.tile([128, HPC * S], BF16, tag="kT")
            vN = qkvp.tile([128, NJT * 512], F16, tag="vN")
            oT = qkvp.tile([128, HPC * NSC * 512], BF16, tag="oT")

            strips = []  # strip tiles by chunk (rotating pool, bufs=2)

            # ---- chunk-0 DMA, kt-piecewise so the PE can start early ----
            strip0 = stripp.tile([128, KT * SCW], BF16, name='strip')
            strips.append(strip0)
            # fine-grained for the first 4 kt (earliest PE start), then
            # 4-kt blocks; rope tables for chunk 0 early, the rest after
            # wv/strip1 (first needed mid-att(0) / during att(0)).
            # (strided half-width weight DMAs measured SLOWER: 512B runs
            # fragment into many packets -- keep contiguous 4-kt blocks)
            nc.sync.dma_start(strip0[:, 0:4 * SCW], hst_d[0][:, 0:4 * SCW])
            nc.sync.dma_start(wq[:, 0:4, :], wq_d[:, 0:4, :])
            nc.sync.dma_start(wk[:, 0:4, :], wk_d[:, 0:4, :])
            nc.sync.dma_start(strip0[:, 4 * SCW:8 * SCW],
                              hst_d[0][:, 4 * SCW:8 * SCW])
            nc.sync.dma_start(wq[:, 4:8, :], wq_d[:, 4:8, :])
            nc.sync.dma_start(wk[:, 4:8, :], wk_d[:, 4:8, :])
            # rope tables fire at ~17us (pass-A end); weight blocks kt4-7
            # are consumed from ~8us -- so tables go after those
            nc.sync.dma_start(cos2[:, 0:SCW], cos_d[:, 0:SCW])
            nc.sync.dma_start(sin2[:, 0:SCW], sin_d[:, 0:SCW])
            nc.sync.dma_start(bmtri[:], bm_d[:])  # att(0) h0 mask ~33us
            for piece in range(2, 4):
                k0 = piece * 4
                nc.sync.dma_start(
                    strip0[:, k0 * SCW:(k0 + 4) * SCW],
                    hst_d[0][:, k0 * SCW:(k0 + 4) * SCW],
                )
                nc.sync.dma_start(wq[:, k0:k0 + 4, :], wq_d[:, k0:k0 + 4, :])
                nc.sync.dma_start(wk[:, k0:k0 + 4, :], wk_d[:, k0:k0 + 4, :])
            nc.sync.dma_start(wv[:], wv_d[:])
            strip1 = stripp.tile([128, KT * SCW], BF16, name='strip')
            strips.append(strip1)
            nc.sync.dma_start(strip1[:], hst_d[1])
            nc.sync.dma_start(cos2[:, SCW:S], cos_d[:, SCW:S])
            nc.sync.dma_start(sin2[:, SCW:S], sin_d[:, SCW:S])
            nc.sync.dma_start(wo[:], wo_d[:])

            # ---- tiny PE warmup: fill the ~3us DMA wait, warm the HAM ----
            wps = avp.tile([128, 512], F32, name='a')
            for _ in range(40):
                nc.tensor.matmul(wps[:, 0:128], warm[:], warm[:],
                                 start=True, stop=True)

            # ================= emission helpers =================

            filler_q: list = []  # list of closures, each emits 1 PE matmul

            def emit_v_group(sc, st):
                """V projection group for jt = 4*sc+st: 16 accumulating
                matmuls + an ACT copy to vN. Emits everything now."""
                strip = strips[sc]
                vp = workp.tile([128, 512], F32, name='w')
                jt = sc * 4 + st
                for kt in range(KT):
                    nc.tensor.matmul(
                        vp[:],
                        strip[:, kt * SCW + st * 128: kt * SCW + (st + 1) * 128],
                        wv[:, kt * 512:(kt + 1) * 512],
                        start=(kt == 0), stop=(kt == KT - 1),
                    )
                # DVE, not ACT: the ACT queue paces attention windows and
                # a copy there delays exps -> AV matmuls wait on vN.
                nc.vector.tensor_copy(vN[:, jt * 512:(jt + 1) * 512], vp[:])

            def push_wo_group(sc, mt, fin_on_act=False):
                """Wo block mt for chunk sc: 4 accumulating matmuls +
                copy + out-DMA, as lazily-allocating filler closures."""
                cell: list = []  # holds fp once the first closure runs
                cl = []
                for h in range(HPC):
                    def mm(h=h, sc=sc, mt=mt, cell=cell):
                        if not cell:
                            cell.append(workp.tile([128, 512], F32, name='w'))
                        nc.tensor.matmul(
                            cell[0][:],
                            wo[:, (h * KT + mt) * 128:(h * KT + mt + 1) * 128],
                            oT[:, (h * NSC + sc) * 512:(h * NSC + sc + 1) * 512],
                            start=(h == 0), stop=(h == HPC - 1),
                        )
                    cl.append(mm)

                def fin(sc=sc, mt=mt, cell=cell, fin_on_act=fin_on_act):
                    fs = fouts.tile([128, 512], BF16, name='fs')
                    # DVE during attention windows (ACT paces them); ACT
                    # for the tail Wo(3), where DVE still has att cleanup
                    # queued and ACT is done with exps
                    if fin_on_act:
                        nc.scalar.copy(fs[:], cell[0][:])
                    else:
                        nc.vector.tensor_copy(fs[:], cell[0][:])
                    # alternate DMA trigger queues: a single queue issues
                    # descriptors at ~600ns each, which bounds the final
                    # out-DMA drain after the last matmul
                    dq = nc.scalar if mt % 2 else nc.sync
                    dq.dma_start(
                        out_d[mt * 128:(mt + 1) * 128, sc * SCW:(sc + 1) * SCW],
                        fs[:],
                    )
                cl.append(fin)
                filler_q.extend(cl)

            def drain_fillers(n):
                for _ in range(n):
                    if filler_q:
                        filler_q.pop(0)()

            def rope(pq, dst, sc):
                """dst = pq*cos + rotate_half(pq)*sin for chunk sc.
                pq: [128,512] PSUM f32; dst: [128,512] SBUF bf16 slice."""
                cs = cos2[:, sc * SCW:(sc + 1) * SCW]
                sn_lo = sin2[0:64, sc * SCW:(sc + 1) * SCW]    # -sin
                sn_hi = sin2[64:128, sc * SCW:(sc + 1) * SCW]  # +sin
                t1 = ropet.tile([128, SCW], F32, name='rt')
                t2 = ropet.tile([128, SCW], F32, name='rt')
                nc.vector.tensor_mul(t1[:], pq[:], cs)
                nc.vector.tensor_mul(t2[0:64, :], pq[64:128, :], sn_lo)
                nc.vector.tensor_mul(t2[64:128, :], pq[0:64, :], sn_hi)
                nc.gpsimd.tensor_add(dst, t1[:], t2[:])

            def attention_head(sc, h, qT, emit_v=False, drain_pat=(2,)):
                """Software-pipelined attention for (chunk sc, head h).
                j-tiles: nfull = 4*sc full [128k,512q] tiles, then 4 band
                tiles with N = 512-128t. Emits fillers between steps.
                emit_v: emit V(sc) group st=j as a block after full step j
                (head 0 of chunks >= 1) so vN is ready for the band."""
                nfull = 4 * sc
                nj = nfull + 4
                qbase = h * SCW

                def qk(j):
                    if j < nfull:
                        n = SCW
                        rhs = qT[:, qbase:qbase + SCW]
                    else:
                        t = j - nfull
                        n = SCW - 128 * t
                        rhs = qT[:, qbase + 128 * t:qbase + SCW]
                    sp = spp.tile([128, SCW], F32, name='s')
                    jj = j  # absolute key tile index == j (tiles 0..nj-1)
                    nc.tensor.matmul(
                        sp[:, 0:n],
                        kTt[:, h * S + jj * 128:h * S + (jj + 1) * 128],
                        rhs, start=True, stop=True,
                    )
                    return sp, n

                av = avp.tile([128, SCW], F32, name='a')
                Ps = pssump.tile([128, SCW], F16, name='Ps')
                sps = {}
                sps[0] = qk(0)
                if nj > 1:
                    sps[1] = qk(1)
                for j in range(nj):
                    sp, n = sps.pop(j)
                    t = j - nfull  # >=0 for band tiles
                    if j == 0:
                        pr = Ps  # exp writes the running-sum tile directly
                        nc.scalar.activation(pr[:, 0:n], sp[:, 0:n], Exp,
                                             scale=float(SCALE))
                    else:
                        pr = probsp.tile([128, SCW], F16, name='pr')
                        nc.scalar.activation(pr[:, 0:n], sp[:, 0:n], Exp,
                                             scale=float(SCALE))
                    if t >= 0:
                        # mask the triangular diagonal sub-block
                        nc.vector.tensor_mul(pr[:, 0:128], pr[:, 0:128],
                                             bmtri[:])
                    if j > 0:
                        c0 = SCW - n
                        nc.vector.tensor_add(Ps[:, c0:SCW], Ps[:, c0:SCW],
                                             pr[:, 0:n])
                    if j + 2 < nj:
                        sps[j + 2] = qk(j + 2)
                    if emit_v and j < 3:
                        # V group j+1 (group j was emitted before this
                        # head's first step / previous iteration)
                        emit_v_group(sc, j + 1)
                    drain_fillers(drain_pat[j % len(drain_pat)])
                    c0 = SCW - n
                    nc.tensor.matmul(
                        av[:, c0:SCW],
                        vN[:, j * 512 + h * 128:j * 512 + (h + 1) * 128],
                        pr[:, 0:n],
                        start=(j == 0), stop=(j == nj - 1),
                    )
                # normalization: partition-sum via ones-matmul (broadcasts
                # the denominator across partitions), then reciprocal.
                drain_fillers(2)
                rs = workp.tile([128, 512], F32, name='w')
                nc.tensor.matmul(rs[:], ones[:], Ps[:], start=True, stop=True)
                rc = recips.tile([128, SCW], F32, name='rc')
                nc.vector.reciprocal_approx_fast(rc[:], rs[:])
                nc.vector.tensor_mul(
                    oT[:, (h * NSC + sc) * 512:(h * NSC + sc + 1) * 512],
                    av[:], rc[:],
                )

            # ================= chunk 0: kt-major Q/K =================
            # Two passes of 4 concurrent PSUM groups: pass-A (heads 0,1)
            # finishes ~halfway so its ropes run on DVE/GpSimd while the
            # PE streams pass-B, instead of one rope burst at the end.
            qT0 = qtp.tile([128, HPC * SCW], BF16, name='qT')
            for half in range(2):
                banks = []
                pools = ((workp, 'w'), (workp, 'w'), (workp, 'w'), (spp, 's')) \
                    if half == 0 else \
                    ((spp, 's'), (spp, 's'), (spp, 's'), (avp, 'a'))
                for pool, nm in pools:
                    banks.append(pool.tile([128, 512], F32, name=nm))
                for kt in range(KT):
                    for i in range(4):
                        h = 2 * half + i // 2
                        wt = wk if i % 2 else wq
                        nc.tensor.matmul(
                            banks[i][:],
                            wt[:, kt, h * 128:(h + 1) * 128],
                            strip0[:, kt * SCW:(kt + 1) * SCW],
                            start=(kt == 0), stop=(kt == KT - 1),
                        )
                for i in range(4):
                    h = 2 * half + i // 2
                    if i % 2:
                        rope(banks[i], kTt[:, h * S:h * S + SCW], 0)
                    else:
                        rope(banks[i], qT0[:, h * SCW:(h + 1) * SCW], 0)

            def push_qk_group(sc, h, isk, qt_cell):
                """Q or K projection group for chunk sc as 16 lazily-
                allocating filler closures + a rope closure."""
                strip = strips[sc]
                cell: list = []
                for kt in range(KT):
                    def mm(kt=kt, h=h, isk=isk, cell=cell, strip=strip):
                        if not cell:
                            cell.append(workp.tile([128, 512], F32, name='w'))
                        wt = wk if isk else wq
                        nc.tensor.matmul(
                            cell[0][:],
                            wt[:, kt, h * 128:(h + 1) * 128],
                            strip[:, kt * SCW:(kt + 1) * SCW],
                            start=(kt == 0), stop=(kt == KT - 1),
                        )
                    filler_q.append(mm)

                def rp(h=h, isk=isk, cell=cell, sc=sc, qt_cell=qt_cell):
                    if isk:
                        rope(cell[0], kTt[:, h * S + sc * SCW:h * S + (sc + 1) * SCW], sc)
                    else:
                        if not qt_cell:
                            qt_cell.append(
                                qtp.tile([128, HPC * SCW], BF16, name='qT'))
                        rope(cell[0], qt_cell[0][:, h * SCW:(h + 1) * SCW], sc)
                filler_q.append(rp)

            def push_chunk_fillers(next_sc, wo_items):
                """Interleave Wo groups (wo_items: list of (sc, mt)),
                QK(next_sc) and V(next_sc) groups in the filler queue:
                [Wog, (Vg), Wog, QKg] x 8 (+ extra Wo at the end). Wo
                first: it gives the prefetched strip DMA a head start
                before QK matmuls (which wait on it) enter the static PE
                stream. V(next_sc) early means its vN copies land well
                before att(next_sc)'s band AV matmuls need them."""
                qt_cell: list = []
                it = iter(wo_items)
                for g in range(8):
                    for pair in (next(it, None),):
                        if pair is not None:
                            push_wo_group(*pair)
                    if next_sc is not None and g < 4:
                        filler_q.append(
                            lambda sc=next_sc, st=g: emit_v_group(sc, st))
                    for pair in (next(it, None),):
                        if pair is not None:
                            push_wo_group(*pair)
                    if next_sc is not None:
                        push_qk_group(next_sc, g // 2, g % 2, qt_cell)
                for pair in it:
                    push_wo_group(*pair)
                return qt_cell

            # att(0), with QK(1) draining as fillers between steps
            qt_next = push_chunk_fillers(1, [])
            for h in range(HPC):
                if h == 0:
                    emit_v_group(0, 0)
                attention_head(0, h, qT0, emit_v=(h == 0))
            drain_fillers(len(filler_q))

            # ================= chunks 1..3 =================
            for sc in range(1, NSC):
                qT = qt_next[0]
                if sc + 1 < NSC:
                    stn = stripp.tile([128, KT * SCW], BF16, name='strip')
                    strips.append(stn)
                    for k0 in range(0, KT, 4):  # piecewise: QK fillers
                        nc.sync.dma_start(     # wait per-piece, not 2MB
                            stn[:, k0 * SCW:(k0 + 4) * SCW],
                            hst_d[sc + 1][:, k0 * SCW:(k0 + 4) * SCW],
                        )
                    wo_items = [(sc - 1, mt) for mt in
                                (range(KT) if sc == 1 else range(KT - 2))]
                    qt_next = push_chunk_fillers(sc + 1, wo_items)
                else:
                    wo_items = ([(sc - 2, KT - 2), (sc - 2, KT - 1)] +
                                [(sc - 1, mt) for mt in range(KT)])
                    push_chunk_fillers(None, wo_items)
                # V(sc) was already pushed/drained during att(sc-1).
                # Last chunk: the queue (only Wo(2), 80 closures) would
                # drain out mid-head-2 at 2/step, leaving head 3 with no
                # PE filler against the ACT-paced exps — ration it.
                for h in range(HPC):
                    # last chunk: only 80 Wo(2) closures for 64 steps --
                    # an even ~1.25/step spread minimizes the worst stall
                    pat = (1, 1, 1, 2) if sc == NSC - 1 else (2,)
                    attention_head(sc, h, qT, drain_pat=pat)
                drain_fillers(len(filler_q))

            # tail: Wo(3) -- fins alternate ACT/DVE (both idle by now)
            for mt in range(KT):
                push_wo_group(NSC - 1, mt, fin_on_act=(mt % 2 == 0))
            drain_fillers(len(filler_q))

    nc.compile()
    return nc


def _build_fallback(mode: str):
    """mode: 'full' | 'general' — the original (baseline) kernel."""
    nc = bacc.Bacc("TRN2", target_bir_lowering=False, debug=False,
                   num_devices=NCORES)

    hst_d = nc.declare_dram_parameter("hst", [NSC, 128, KT * SCW], BF16, isOutput=False)
    wq_d = nc.declare_dram_parameter("wq", [128, HPC * KT * 128], BF16, isOutput=False)
    wk_d = nc.declare_dram_parameter("wk", [128, HPC * KT * 128], BF16, isOutput=False)
    wv_d = nc.declare_dram_parameter("wv", [128, KT * 512], BF16, isOutput=False)
    wo_d = nc.declare_dram_parameter("wo", [128, HPC * KT * 128], BF16, isOutput=False)
    cos_d = nc.declare_dram_parameter("cost", [64, S], F32, isOutput=False)
    sin_d = nc.declare_dram_parameter("sint", [64, S], F32, isOutput=False)
    if mode == "general":
        em_d = nc.declare_dram_parameter("emask", [S, S], F16, isOutput=False)
    out_d = nc.declare_dram_parameter("outT", [HDIM, S], F32, isOutput=True)

    with tile.TileContext(nc) as tc:
        with (
            tc.tile_pool(name="wpool", bufs=1) as wpool,
            tc.tile_pool(name="cpool", bufs=1) as cpool,
            tc.tile_pool(name="qkv", bufs=1) as qkvp,
            tc.tile_pool(name="strip", bufs=2) as stripp,
            tc.tile_pool(name="ropet", bufs=2) as ropet,
            tc.tile_pool(name="probs", bufs=4) as probsp,
            tc.tile_pool(name="psums", bufs=2) as psums,
            tc.tile_pool(name="recips", bufs=2) as recips,
            tc.tile_pool(name="fouts", bufs=3) as fouts,
            tc.tile_pool(name="emt", bufs=4) as emtp,
            tc.tile_pool(name="pp", bufs=2, space="PSUM") as pp_pool,
            tc.tile_pool(name="sp", bufs=2, space="PSUM") as sp_pool,
            tc.tile_pool(name="av", bufs=2, space="PSUM") as av_pool,
            tc.tile_pool(name="misc_ps", bufs=2, space="PSUM") as misc_ps,
        ):
            wq = wpool.tile([128, HPC * KT * 128], BF16, tag="wq")
            wk = wpool.tile([128, HPC * KT * 128], BF16, tag="wk")
            wv = wpool.tile([128, KT * 512], BF16, tag="wv")
            wo = wpool.tile([128, HPC * KT * 128], BF16, tag="wo")
            cos = cpool.tile([64, S], F32, tag="cos")
            sin = cpool.tile([64, S], F32, tag="sin")
            ones = cpool.tile([128, 128], F16, tag="ones")
            nc.gpsimd.memset(ones[:], 1.0)

            qT = qkvp.tile([128, HPC * S], BF16, tag="qT")
            kTt = qkvp.tile([128, HPC * S], BF16, tag="kT")
            vN = qkvp.tile([128, NJT * 512], F16, tag="vN")
            oT = qkvp.tile([128, HPC * NSC * 512], BF16, tag="oT")

            warm = cpool.tile([128, 512], BF16, tag="warm")
            nc.gpsimd.memset(warm[:, 0:128], 0.0)
            wps = misc_ps.tile([128, SCW], F32, tag="mp")
            for _ in range(100):
                nc.tensor.matmul(wps[:], warm[:, 0:128], warm[:],
                                 start=True, stop=True)

            for sc in range(NSC):
                strip = stripp.tile([128, KT * SCW], BF16)
                nc.sync.dma_start(strip[:], hst_d[sc])
                if sc == 0:
                    nc.sync.dma_start(wq[:], wq_d[:])
                    nc.sync.dma_start(cos[:], cos_d[:])
                    nc.sync.dma_start(sin[:], sin_d[:])
                    nc.sync.dma_start(wk[:], wk_d[:])
                    nc.sync.dma_start(wv[:], wv_d[:])
                    nc.sync.dma_start(wo[:], wo_d[:])
                cs = cos[:, sc * SCW:(sc + 1) * SCW]
                sn = sin[:, sc * SCW:(sc + 1) * SCW]
                for h in range(HPC):
                    for wt, dst in ((wq, qT), (wk, kTt)):
                        pq = pp_pool.tile([128, SCW], F32, tag="pp")
                        for kt in range(KT):
                            nc.tensor.matmul(
                                pq[:],
                                wt[:, (h * KT + kt) * 128:(h * KT + kt + 1) * 128],
                                strip[:, kt * SCW:(kt + 1) * SCW],
                                start=(kt == 0), stop=(kt == KT - 1),
                            )
                        dlo = dst[0:64, h * S + sc * SCW: h * S + (sc + 1) * SCW]
                        dhi = dst[64:128, h * S + sc * SCW: h * S + (sc + 1) * SCW]
                        t1 = ropet.tile([128, SCW], F32, tag="t1")
                        t2 = ropet.tile([128, SCW], F32, tag="t2")
                        nc.vector.tensor_mul(t1[0:64, :], pq[0:64, :], cs)
                        nc.vector.tensor_mul(t1[64:128, :], pq[64:128, :], cs)
                        nc.vector.tensor_mul(t2[0:64, :], pq[64:128, :], sn)
                        nc.vector.tensor_mul(t2[64:128, :], pq[0:64, :], sn)
                        nc.vector.tensor_sub(dlo, t1[0:64, :], t2[0:64, :])
                        nc.vector.tensor_add(dhi, t1[64:128, :], t2[64:128, :])
                for st in range(4):
                    vp = pp_pool.tile([128, SCW], F32, tag="pp")
                    for kt in range(KT):
                        nc.tensor.matmul(
                            vp[:],
                            strip[:, kt * SCW + st * 128: kt * SCW + (st + 1) * 128],
                            wv[:, kt * 512:(kt + 1) * 512],
                            start=(kt == 0), stop=(kt == KT - 1),
                        )
                    jt = sc * 4 + st
                    nc.scalar.copy(vN[:, jt * 512:(jt + 1) * 512], vp[:])

                ics = list(range(NSC)) if sc == NSC - 1 else []
                for ic in ics:
                    nj = NJT
                    for h in range(HPC):
                        av = av_pool.tile([128, SCW], F32)
                        Ps = psums.tile([128, SCW], F16)
                        for j in range(nj):
                            sp = sp_pool.tile([128, SCW], F32)
                            nc.tensor.matmul(
                                sp[:],
                                kTt[:, h * S + j * 128: h * S + (j + 1) * 128],
                                qT[:, h * S + ic * SCW: h * S + (ic + 1) * SCW],
                                start=True, stop=True,
                            )
                            pr = probsp.tile([128, SCW], F16)
                            nc.scalar.activation(pr[:], sp[:], Exp, scale=float(SCALE))
                            if mode == "general":
                                emt = emtp.tile([128, SCW], F16)
                                nc.sync.dma_start(
                                    emt[:],
                                    em_d[j * 128:(j + 1) * 128, ic * SCW:(ic + 1) * SCW],
                                )
                                nc.vector.tensor_mul(pr[:], pr[:], emt[:])
                            if j == 0:
                                nc.vector.tensor_copy(Ps[:], pr[:])
                            else:
                                nc.vector.tensor_add(Ps[:], Ps[:], pr[:])
                            nc.tensor.matmul(
                                av[:],
                                vN[:, j * 512 + h * 128: j * 512 + (h + 1) * 128],
                                pr[:],
                                start=(j == 0), stop=(j == nj - 1),
                            )
                        rs = misc_ps.tile([128, SCW], F32, tag="mp")
                        nc.tensor.matmul(rs[:], ones[:], Ps[:], start=True, stop=True)
                        rc = recips.tile([128, SCW], F32)
                        nc.vector.reciprocal_approx_fast(rc[:], rs[:])
                        nc.vector.tensor_mul(
                            oT[:, (h * NSC + ic) * 512:(h * NSC + ic + 1) * 512],
                            av[:], rc[:],
                        )
                    for mt in range(KT):
                        fp = misc_ps.tile([128, SCW], F32, tag="mp")
                        for h in range(HPC):
                            nc.tensor.matmul(
                                fp[:],
                                wo[:, (h * KT + mt) * 128:(h * KT + mt + 1) * 128],
                                oT[:, (h * NSC + ic) * 512:(h * NSC + ic + 1) * 512],
                                start=(h == 0), stop=(h == HPC - 1),
                            )
                        fs = fouts.tile([128, SCW], F32)
                        nc.scalar.copy(fs[:], fp[:])
                        nc.sync.dma_start(
                            out_d[mt * 128:(mt + 1) * 128, ic * SCW:(ic + 1) * SCW],
                            fs[:],
                        )

    nc.compile()
    return nc


def _get_nc(mode: str):
    if mode not in _NC_CACHE:
        if mode == "causal":
            _NC_CACHE[mode] = _build_causal()
        else:
            _NC_CACHE[mode] = _build_fallback(mode)
    return _NC_CACHE[mode]


def _classify_mask(m: np.ndarray) -> str:
    if not m.any():
        return "full"
    tril = np.tril(np.ones((S, S), dtype=bool))
    if np.all(m[tril] == 0.0) and np.all(m[~tril] <= -1e8):
        return "causal"
    return "general"


def kernel(hidden_states, attention_mask, position_ids, Wq, Wk, Wv, Wo):
    global LAST_EXEC_TIME_NS
    hs = np.asarray(hidden_states, dtype=np.float32)
    mask = np.asarray(attention_mask, dtype=np.float32)[0, 0]
    pos = np.asarray(position_ids)
    Wq = np.asarray(Wq, dtype=np.float32)
    Wk = np.asarray(Wk, dtype=np.float32)
    Wv = np.asarray(Wv, dtype=np.float32)
    Wo = np.asarray(Wo, dtype=np.float32)

    mode = _classify_mask(mask)
    nc = _get_nc(mode)

    inv_freq = 1.0 / (ROPE_BASE ** (np.arange(0, HD, 2, dtype=np.float32) / HD))
    cos_b, sin_b = [], []
    for b in range(B):
        ang = np.outer(pos[b].astype(np.float32), inv_freq)  # [S, 64]
        cos_b.append(np.cos(ang).T.astype(np.float32).copy())  # [64, S]
        sin_b.append(np.sin(ang).T.astype(np.float32).copy())

    in_maps = []
    for c in range(NCORES):
        b = c // CPB
        r0 = (c % CPB) * HPC * HD

        hsb = hs[b]  # [S, HDIM]
        hst = (hsb.reshape(NSC, SCW, KT, 128).transpose(0, 3, 2, 1)
               .reshape(NSC, 128, KT * SCW).astype(NPBF16))

        Wv_s = Wv[r0:r0 + 512]
        wv_t = (Wv_s.reshape(512, KT, 128).transpose(2, 1, 0)
                .reshape(128, KT * 512).astype(NPBF16))
        Wo_s = Wo[:, r0:r0 + 512]
        wo_t = (Wo_s.reshape(KT, 128, HPC, 128).transpose(3, 2, 0, 1)
                .reshape(128, HPC * KT * 128).astype(NPBF16))

        if mode == "causal":
            # flat kt-major weight layout [ki, kt*512 + h*128 + fo] —
            # exactly the SBUF tile layout, so DMA slices are contiguous
            Wq_s = Wq[r0:r0 + 512]
            wq_t = np.ascontiguousarray(
                Wq_s.reshape(HPC, 128, KT, 128).transpose(3, 2, 0, 1)
                .reshape(128, KT, 512)).astype(NPBF16)
            Wk_s = Wk[r0:r0 + 512]
            wk_t = np.ascontiguousarray(
                Wk_s.reshape(HPC, 128, KT, 128).transpose(3, 2, 0, 1)
                .reshape(128, KT, 512)).astype(NPBF16)
            cos2 = np.concatenate([cos_b[b], cos_b[b]], axis=0)  # [128, S]
            sin2 = np.concatenate([-sin_b[b], sin_b[b]], axis=0)
            pidx = np.arange(128)[:, None]
            xidx = np.arange(128)[None, :]
            bmtri = (pidx <= xidx).astype(NPF16)
            m = {
                "hst": hst, "wq": wq_t, "wk": wk_t, "wv": wv_t, "wo": wo_t,
                "cos2": np.ascontiguousarray(cos2),
                "sin2": np.ascontiguousarray(sin2),
                "bmtri": bmtri,
            }
        else:
            Wq_s = Wq[r0:r0 + 512]
            wq_t = (Wq_s.reshape(HPC, 128, KT, 128).transpose(3, 0, 2, 1)
                    .reshape(128, HPC * KT * 128).astype(NPBF16))
            Wk_s = Wk[r0:r0 + 512]
            wk_t = (Wk_s.reshape(HPC, 128, KT, 128).transpose(3, 0, 2, 1)
                    .reshape(128, HPC * KT * 128).astype(NPBF16))
            m = {
                "hst": hst, "wq": wq_t, "wk": wk_t, "wv": wv_t, "wo": wo_t,
                "cost": cos_b[b], "sint": sin_b[b],
            }
            if mode == "general":
                with np.errstate(under="ignore", over="ignore"):
                    m["emask"] = np.exp(mask.T.astype(np.float64)).astype(NPF16)
        in_maps.append(m)

    trace = os.environ.get("BASS_KERNEL_TRACE") == "1"
    res = run_bass_kernel_spmd(nc, in_maps, core_ids=list(range(NCORES)),
                               trace=trace)
    LAST_EXEC_TIME_NS = res.exec_time_ns

    out = np.empty((B, S, HDIM), dtype=np.float32)
    for b in range(B):
        acc = res.results[CPB * b]["outT"].astype(np.float32)
        for c in range(CPB * b + 1, CPB * (b + 1)):
            acc = acc + res.results[c]["outT"].astype(np.float32)
        out[b] = acc.T
    return out
